# revision 11
# baseline (speedup 1.0000x reference)
"""GQA causal attention (B=2, T=2048, C=2048, 32 Q heads, 8 KV heads) on 8
Trainium2 NeuronCores — v6.

Sharding: tensor-parallel over KV-head groups for projections+attention
(core i owns KV head i and its 4 query heads). Output projection is
row-sharded with INTERLEAVED ownership: for each exchange unit
u = (batch b, chunk-pair p) covering t-chunks qc=2p,2p+1, core i owns the
i-th 64-t slice of each chunk. One AllToAll per unit (4 total, 512 KiB
per core each) so only the last unit's exchange + one 128-row out-proj
sit after the final attention chunk. Collectives measured free on HW
(fully overlapped).

v6 structure (attention on HW is Act-engine bound: exp costs
(N+352)/1.2 ns, i.e. 720 ns per [128,512] block vs 852 ns of PE work
per 2-head iteration):
  - both heads of a pair score into ONE 2-bank PSUM tile [128,2,512];
    a single activation with a 3D AP exps both (N=2*fr amortizes the
    352-cycle fixed cost): Act total 210 -> ~163 us, under PE's ~270;
  - pipelined inner loop: iteration kb emits scores(kb) (gated on
    exp(kb-1) freeing the score tile) then AVs(kb-1) from the previous
    ex tile, so the PE never waits exp latency in-line;
  - FILLER injection: next chunk's projection groups and ready
    out-projection occ-groups are injected into attention iterations,
    giving the PE work while Act catches up and covering the thin
    diagonal blocks;
  - AV matmuls trimmed to the causal range [qoff:] (no memsets);
  - per-head normalization: recip + PSUM->SBUF copy at pair end frees
    the y bank; broadcast matmul + multiply + ytl DMA deferred into the
    next pair's first iterations;
  - x chunk prefetched one chunk ahead (4 split DMAs on the SP ring);
    weights ride the Act HWDGE ring; yts staging on the gpsimd SWDGE
    ring so collective-dependent DMAs never head-block the SP ring.
PSUM banks: pp(2, proj+outproj) + sps(1x2banks, scores) + yps(3, y
accum) + bps(1, 1/l broadcast) = 8.
"""

import sys

sys.path.insert(0, "/opt/trn_rl_repo")

import numpy as np
import ml_dtypes

import concourse.bass as bass
import concourse.mybir as mybir
import concourse.tile as tile

P = 128
B, T, C = 2, 2048, 2048
BT = B * T            # 4096
NH, NKV = 32, 8
HD = C // NH          # 64
G = NH // NKV         # 4 q heads per kv head / per core
CQ = G * HD           # 256 q channels per core
KC = C // P           # 16 contraction chunks
TQ = 512              # t-chunk
NCORES = 8
NTB = BT // TQ        # 8 chunks; chunk tb has (b, qc) = (tb//4, tb%4)
NU = 4                # exchange units: u = 2*b + p, chunk-pair p
UT = P                # t rows per core per unit (64 from each chunk)

f32 = mybir.dt.float32
f32r = mybir.dt.float32r
bf16 = mybir.dt.bfloat16
EXP = mybir.ActivationFunctionType.Exp
SCALE = float(HD) ** -0.5


def split_multi_waits(nc):
    """Walrus codegen allows only one sync-wait per engine instruction; move
    extras onto standalone same-engine EventSemaphore waits placed before."""
    for fn in nc.m.functions:
        for bb in fn.blocks:
            out = []
            for inst in bb.instructions:
                si = inst.sync_info
                if si is not None and si.on_wait and len(si.on_wait) > 1:
                    waits = list(si.on_wait)
                    for j, w in enumerate(waits[:-1]):
                        nop = mybir.InstEventSemaphore(
                            name=f"{inst.name}-ws{j}", ins=[], outs=[],
                            engine=inst.engine)
                        nop.sync_info = mybir.SyncInfo(on_wait=[w], on_update=[])
                        out.append(nop)
                    inst.sync_info = mybir.SyncInfo(
                        on_wait=[waits[-1]], on_update=list(si.on_update))
                out.append(inst)
            try:
                bb.instructions[:] = out
            except TypeError:
                bb.instructions.clear()
                bb.instructions.extend(out)


def build(reps=1, split=True, variant="full"):
    """variant: 'full' | 'nocc' (collectives skipped — timing-only)."""
    nc = bass.Bass(num_devices=NCORES)

    xt_d = nc.dram_tensor("xt", [C, BT], bf16, kind="ExternalInput")
    wq_d = nc.dram_tensor("wq", [C, CQ], bf16, kind="ExternalInput")
    wkv_d = nc.dram_tensor("wkv", [C, P], bf16, kind="ExternalInput")
    wp_d = nc.dram_tensor("wp", [C, C], bf16, kind="ExternalInput")
    bpb_d = nc.dram_tensor("bpb", [P, C], f32, kind="ExternalInput")
    mask_d = nc.dram_tensor("masks", [P, P], bf16, kind="ExternalInput")
    ones_d = nc.dram_tensor("ones", [1, HD], f32r, kind="ExternalInput")
    idn_d = nc.dram_tensor("ident", [P, P], f32, kind="ExternalInput")
    vpad_d = nc.dram_tensor("vpad", [P, (TQ // P) * 2], bf16,
                            kind="ExternalInput")
    out_d = nc.dram_tensor("out", [NU * UT, C], f32, kind="ExternalOutput")

    xt_v = xt_d.rearrange("(o p) n -> p o n", p=P)
    wq_v = wq_d.rearrange("(o p) n -> p o n", p=P)
    wkv_v = wkv_d.rearrange("(o p) n -> p o n", p=P)
    wp_v = wp_d.rearrange("(o p) n -> p o n", p=P)

    with tile.TileContext(nc) as tc:
      for rep in range(reps):
        with tc.tile_pool(name=f"res{rep}", bufs=1) as res, \
             tc.tile_pool(name=f"dram{rep}", bufs=1, space="DRAM") as dp:
            wq_sb = res.tile([P, KC, CQ], bf16, name=f"wq{rep}")
            wkv_sb = res.tile([P, KC, P], bf16, name=f"wkv{rep}")
            wp_sb = res.tile([P, KC, C], bf16, name=f"wp{rep}")
            bpb_sb = res.tile([P, C], f32, name=f"bp{rep}")
            idn_sb = res.tile([P, P], f32, name=f"idn{rep}")
            mask_sb = res.tile([P, P], bf16, name=f"mk{rep}")
            ones_sb = res.tile([1, HD], f32r, name=f"on{rep}")

            # weights & consts on the Act HWDGE ring (SP ring carries x)
            for hh in range(2):
                nc.scalar.dma_start(wq_sb[:, hh * 8:(hh + 1) * 8, :],
                                    wq_v[:, hh * 8:(hh + 1) * 8, :])
            nc.scalar.dma_start(wkv_sb[:], wkv_v[:, :, :])
            nc.scalar.dma_start(idn_sb[:], idn_d[:, :])
            nc.scalar.dma_start(mask_sb[:], mask_d[:, :])
            nc.scalar.dma_start(ones_sb[:], ones_d[:, :])

            qT = [[res.tile([HD, TQ], bf16, name=f"q{rep}_{h}_{tb}")
                   for tb in range(NTB)] for h in range(G)]
            kTt = [res.tile([HD, TQ], bf16, name=f"k{rep}_{tb}")
                   for tb in range(NTB)]
            va_t = [res.tile([P, TQ // P, HD + 2], bf16, name=f"v{rep}_{tb}")
                    for tb in range(NTB)]
            for tb in range(NTB):
                nc.scalar.dma_start(
                    va_t[tb][:, :, HD:HD + 2],
                    vpad_d.rearrange("p (k t) -> p k t", t=2))

            ytl = [dp.tile([NCORES * CQ, UT], bf16, name=f"ytl{rep}_{u}")
                   for u in range(NU)]
            yta = [dp.tile([NCORES * CQ, UT], bf16, name=f"yta{rep}_{u}")
                   for u in range(NU)]

            with tc.tile_pool(name=f"xp{rep}", bufs=2) as xp, \
                 tc.tile_pool(name=f"pp{rep}", bufs=2, space="PSUM") as pp, \
                 tc.tile_pool(name=f"sps{rep}", bufs=1, space="PSUM") as sps, \
                 tc.tile_pool(name=f"yps{rep}", bufs=3, space="PSUM") as yps, \
                 tc.tile_pool(name=f"bps{rep}", bufs=1, space="PSUM") as bps, \
                 tc.tile_pool(name=f"ep{rep}", bufs=3) as ep, \
                 tc.tile_pool(name=f"np{rep}", bufs=3) as npo, \
                 tc.tile_pool(name=f"fp{rep}", bufs=2) as fp, \
                 tc.tile_pool(name=f"vp{rep}", bufs=2) as vp:
                yts_tiles = {}
                xtiles = {}
                pending = []   # deferred norm closures
                fillers = []   # list of (weight, closure): 'light'|'heavy'

                def take_filler(light_only):
                    for i, (w, f) in enumerate(fillers):
                        if not light_only or w == "light":
                            fillers.pop(i)
                            f()
                            return True
                    return False

                def flush_fillers():
                    while fillers:
                        _, f = fillers.pop(0)
                        f()

                def flush_pending():
                    for _, f in pending:
                        f()
                    pending.clear()

                def prefetch_xt(tb):
                    xt_t = xp.tile([P, KC, TQ], bf16, tag="xt",
                                   name=f"xt{rep}_{tb}")
                    for i in range(4):
                        nc.sync.dma_start(
                            xt_t[:, 4 * i:4 * i + 4, :],
                            xt_v[:, 4 * i:4 * i + 4, tb * TQ:(tb + 1) * TQ])
                    xtiles[tb] = xt_t

                def proj_units(tb):
                    """Filler units computing chunk tb's q/k/v projections."""
                    xt_t = xtiles[tb]

                    def qu(half):
                        def f():
                            ps = pp.tile([P, TQ], f32, tag="pp",
                                         name=f"pq{rep}")
                            for c in range(KC):
                                nc.tensor.matmul(
                                    ps[:],
                                    wq_sb[:, c, half * P:(half + 1) * P],
                                    xt_t[:, c, :],
                                    start=(c == 0), stop=(c == KC - 1))
                            nc.vector.tensor_copy(qT[2 * half][tb][:],
                                                  ps[0:HD, :])
                            nc.vector.tensor_copy(qT[2 * half + 1][tb][:],
                                                  ps[HD:P, :])
                        return f

                    vs_box = [None]

                    def kvu():
                        ps = pp.tile([P, TQ], f32, tag="pp", name=f"pk{rep}")
                        for c in range(KC):
                            nc.tensor.matmul(ps[:], wkv_sb[:, c, :],
                                             xt_t[:, c, :],
                                             start=(c == 0),
                                             stop=(c == KC - 1))
                        nc.vector.tensor_copy(kTt[tb][:], ps[0:HD, :])
                        vs = vp.tile([HD, TQ], f32, tag="vs", name=f"vs{rep}")
                        nc.vector.tensor_copy(vs[:], ps[HD:P, :])
                        vs_box[0] = vs

                    def vtu():
                        vs = vs_box[0]
                        for k4 in range(TQ // P):
                            vt_ps = pp.tile([P, HD], f32, tag="pp",
                                            name=f"vt{rep}")
                            nc.tensor.transpose(vt_ps[:],
                                                vs[:, k4 * P:(k4 + 1) * P],
                                                idn_sb[0:HD, 0:HD])
                            nc.vector.tensor_copy(va_t[tb][:, k4, 0:HD],
                                                  vt_ps[:])

                    return [("heavy", qu(0)), ("heavy", qu(1)),
                            ("heavy", kvu), ("heavy", vtu)]

                def emit_norm(b, qc, hp, y0, y1):
                    """recip + PSUM->SBUF copies now (frees the y banks);
                    bc matmul + normalize mul + ytl dma per head deferred."""
                    h0, h1 = 2 * hp, 2 * hp + 1
                    u, c2 = 2 * b + qc // 2, qc % 2
                    rys = []
                    for hi, y_ps in ((0, y0), (1, y1)):
                        rr = npo.tile([1, TQ], f32r, tag=f"rr{hi}",
                                      name=f"rr{rep}")
                        with nc.allow_low_precision(
                                reason="1/l in f32r (22-bit) is plenty"):
                            nc.vector.reciprocal(rr[:], y_ps[HD:HD + 1, :])
                        ys = npo.tile([HD, TQ], bf16, tag=f"ys{hi}",
                                      name=f"ys{rep}")
                        nc.vector.tensor_copy(ys[:], y_ps[0:HD, :])
                        rys.append((rr, ys))

                    def do_head(hi):
                        rr, ys = rys[hi]
                        h = h0 if hi == 0 else h1
                        bc = bps.tile([HD, TQ], f32, tag="bc",
                                      name=f"bc{rep}")
                        nc.tensor.matmul(bc[:], ones_sb[:], rr[:],
                                         start=True, stop=True)
                        yn = npo.tile([HD, TQ], bf16, tag=f"yn{hi}",
                                      name=f"yn{rep}")
                        nc.vector.tensor_mul(yn[:], ys[:], bc[:])
                        dst = ytl[u].rearrange(
                            "(j ch) (c2 t) -> ch j c2 t", j=NCORES, c2=2)
                        nc.sync.dma_start(
                            dst[h * HD:(h + 1) * HD, :, c2, :],
                            yn[:].rearrange("d (j t) -> d j t", j=NCORES))

                    pending.append(("h0", lambda: do_head(0)))
                    pending.append(("h1", lambda: do_head(1)))

                def emit_exchange(u):
                    if variant != "nocc":
                        nc.gpsimd.collective_compute(
                            "AllToAll", mybir.AluOpType.bypass,
                            replica_groups=[list(range(NCORES))],
                            ins=[ytl[u][:].opt()], outs=[yta[u][:].opt()])
                    yts = fp.tile([P, KC, UT], bf16, tag="yt",
                                  name=f"yt{rep}")
                    nc.gpsimd.dma_start(
                        yts[:], yta[u].rearrange("(c p) t -> p c t", p=P))
                    yts_tiles[u] = yts

                def outproj_units(u):
                    yts = yts_tiles.pop(u)

                    def ou(occ):
                        def f():
                            o_ps = pp.tile([P, TQ], f32, tag="pp",
                                           name=f"o{rep}_{u}_{occ}")
                            for c in range(KC):
                                nc.tensor.matmul(
                                    o_ps[:], yts[:, c, :],
                                    wp_sb[:, c, occ * TQ:(occ + 1) * TQ],
                                    start=(c == 0), stop=(c == KC - 1))
                            o_sb = fp.tile([P, TQ], f32, tag="ob",
                                           name=f"ob{rep}")
                            nc.vector.tensor_add(
                                o_sb[:], o_ps[:],
                                bpb_sb[:, occ * TQ:(occ + 1) * TQ])
                            nc.sync.dma_start(
                                out_d[u * P:(u + 1) * P,
                                      occ * TQ:(occ + 1) * TQ],
                                o_sb[:])
                        return f

                    return [("light", ou(occ)) for occ in range(4)]

                # ---------------- chunk loop ----------------
                prefetch_xt(0)
                for tb in range(NTB):
                    b, qc = tb // 4, tb % 4
                    if tb + 1 < NTB:
                        prefetch_xt(tb + 1)
                    if tb == 0:
                        # chunk 0's projections run inline (nothing to
                        # overlap them with yet)
                        for _, f in proj_units(0):
                            f()
                    if tb + 1 < NTB:
                        fillers.extend(proj_units(tb + 1))
                    if tb == 3:
                        fillers.extend(outproj_units(0))
                    elif tb == 5:
                        fillers.extend(outproj_units(1))
                    elif tb == 7:
                        fillers.extend(outproj_units(2))

                    # ---- attention: head pairs, merged-exp pipeline ----
                    nkb = 4 * qc + 4
                    for hp in range(2):
                        h0, h1 = 2 * hp, 2 * hp + 1
                        y0 = yps.tile([HD + 2, TQ], f32, tag="y",
                                      name=f"y{rep}_{tb}_{h0}")
                        y1 = yps.tile([HD + 2, TQ], f32, tag="y",
                                      name=f"y{rep}_{tb}_{h1}")
                        prev = [None]   # (ex2, kb, qoff)

                        def emit_avs(y0=y0, y1=y1, nkb=nkb, b=b, prev=prev):
                            ex2, kb, qoff = prev[0]
                            tb_k = b * 4 + kb // 4
                            for hi, y_ps in ((0, y0), (1, y1)):
                                nc.tensor.matmul(
                                    y_ps[:, qoff:TQ],
                                    va_t[tb_k][:, kb % 4, :],
                                    ex2[:, hi, qoff:TQ],
                                    start=(kb == 0), stop=(kb == nkb - 1))
                            prev[0] = None

                        for kb in range(nkb):
                            j = kb - 4 * qc
                            qoff = max(0, j * P)
                            fr = TQ - qoff
                            tb_k = b * 4 + kb // 4
                            s2 = sps.tile([P, 2, TQ], f32, tag="s2",
                                          name=f"s{rep}")
                            for hi, h in ((0, h0), (1, h1)):
                                nc.tensor.matmul(
                                    s2[:, hi, 0:fr],
                                    kTt[tb_k][:, (kb % 4) * P:
                                              (kb % 4 + 1) * P],
                                    qT[h][tb][:, qoff:TQ],
                                    start=True, stop=True)
                            ex2 = ep.tile([P, 2, TQ], bf16, tag="ex",
                                          name=f"ex{rep}")
                            nc.scalar.activation(ex2[:, :, qoff:TQ],
                                                 s2[:, :, 0:fr], EXP,
                                                 scale=SCALE)
                            if j >= 0:
                                for hi in range(2):
                                    nc.vector.tensor_mul(
                                        ex2[:, hi, qoff:qoff + P],
                                        ex2[:, hi, qoff:qoff + P],
                                        mask_sb[:])
                            # norm-of-prev-pair injections (y-bank order)
                            if kb == 0 and pending:
                                pending[0][1]()
                                del pending[0]
                            if kb == 1 and pending:
                                flush_pending()
                            if prev[0] is not None:
                                emit_avs()
                            # filler slot
                            it_global = hp * nkb + kb
                            can_heavy = (hp == 1) or \
                                (it_global >= max(4, nkb // 2))
                            if j >= 1:
                                take_filler(light_only=not (qc == 0 and
                                                            can_heavy))
                            elif kb % 2 == 0:
                                if can_heavy:
                                    take_filler(light_only=False)
                                else:
                                    take_filler(light_only=True)
                            prev[0] = (ex2, kb, qoff)
                        emit_avs()
                        emit_norm(b, qc, hp, y0, y1)

                    flush_fillers()
                    # wp spread over chunks 0-1 on the Act ring (out-proj
                    # of unit 0 consumes it from chunk 3)
                    if tb < 2:
                        for ww in range(2):
                            wc = 8 * tb + 4 * ww
                            nc.scalar.dma_start(
                                wp_sb[:, wc:wc + 4, :],
                                wp_v[:, wc:wc + 4, :])
                    if tb == 1:
                        nc.scalar.dma_start(bpb_sb[:], bpb_d[:, :])

                    # ---- unit boundaries ----
                    if tb % 2 == 1:
                        flush_pending()
                        emit_exchange(2 * b + qc // 2)
                    if tb == 7:
                        flush_fillers()
                        for _, f in outproj_units(3):
                            f()

    if split:
        split_multi_waits(nc)
    return nc


_NC_CACHE = None


def _get_nc():
    global _NC_CACHE
    if _NC_CACHE is None:
        _NC_CACHE = build()
    return _NC_CACHE


def make_in_maps(x, wq, wk, wv, wp, bp):
    x = np.asarray(x, dtype=np.float32)
    xt = np.ascontiguousarray(x.reshape(BT, C).T).astype(ml_dtypes.bfloat16)
    wp_b = np.ascontiguousarray(np.asarray(wp, np.float32)).astype(
        ml_dtypes.bfloat16)
    bpb = np.tile(np.asarray(bp, np.float32)[None, :], (P, 1))
    mask = np.triu(np.ones((P, P), np.float32)).astype(ml_dtypes.bfloat16)
    ident = np.eye(P, dtype=np.float32)
    vpad = np.zeros((P, TQ // P, 2), np.float32)
    vpad[:, :, 0] = 1.0
    vpad = vpad.reshape(P, -1).astype(ml_dtypes.bfloat16)
    in_maps = []
    for i in range(NCORES):
        cs = slice(i * CQ, (i + 1) * CQ)
        ks = slice(i * HD, (i + 1) * HD)
        wkv = np.concatenate(
            [np.asarray(wk, np.float32)[:, ks],
             np.asarray(wv, np.float32)[:, ks]], axis=1)
        in_maps.append({
            "xt": xt,
            "wq": np.ascontiguousarray(
                np.asarray(wq, np.float32)[:, cs]).astype(ml_dtypes.bfloat16),
            "wkv": np.ascontiguousarray(wkv).astype(ml_dtypes.bfloat16),
            "wp": wp_b,
            "bpb": bpb,
            "masks": mask,
            "ones": np.ones((1, HD), np.float32),
            "ident": ident,
            "vpad": vpad,
        })
    return in_maps


def kernel(x, wq, wk, wv, wp, bp, _trace=False):
    from concourse.bass_utils import run_bass_kernel_spmd
    nc = _get_nc()
    in_maps = make_in_maps(x, wq, wk, wv, wp, bp)
    res = run_bass_kernel_spmd(nc, in_maps, list(range(NCORES)), trace=_trace)
    out = np.empty((B, T, C), np.float32)
    for i in range(NCORES):
        o = res.results[i]["out"]       # [NU*UT, C]
        for u in range(NU):
            bb, p = u // 2, u % 2
            for c2 in range(2):
                t0 = (2 * p + c2) * TQ + i * HD
                out[bb, t0:t0 + HD, :] = \
                    o[u * P + c2 * HD:u * P + (c2 + 1) * HD, :]
    if _trace:
        return out, res
    return out


# revision 12
# speedup vs baseline: 1.4410x; 1.4410x over previous
"""GQA causal attention (B=2, T=2048, C=2048, 32 Q heads, 8 KV heads) on 8
Trainium2 NeuronCores — v6.

Sharding: tensor-parallel over KV-head groups for projections+attention
(core i owns KV head i and its 4 query heads). Output projection is
row-sharded with INTERLEAVED ownership: for each exchange unit
u = (batch b, chunk-pair p) covering t-chunks qc=2p,2p+1, core i owns the
i-th 64-t slice of each chunk. One AllToAll per unit (4 total, 512 KiB
per core each) so only the last unit's exchange + one 128-row out-proj
sit after the final attention chunk. Collectives measured free on HW
(fully overlapped).

v6 structure (attention on HW is Act-engine bound: exp costs
(N+352)/1.2 ns, i.e. 720 ns per [128,512] block vs 852 ns of PE work
per 2-head iteration):
  - both heads of a pair score into ONE 2-bank PSUM tile [128,2,512];
    a single activation with a 3D AP exps both (N=2*fr amortizes the
    352-cycle fixed cost): Act total 210 -> ~163 us, under PE's ~270;
  - pipelined inner loop: iteration kb emits scores(kb) (gated on
    exp(kb-1) freeing the score tile) then AVs(kb-1) from the previous
    ex tile, so the PE never waits exp latency in-line;
  - FILLER injection: next chunk's projection groups and ready
    out-projection occ-groups are injected into attention iterations,
    giving the PE work while Act catches up and covering the thin
    diagonal blocks;
  - AV matmuls trimmed to the causal range [qoff:] (no memsets);
  - per-head normalization: recip + PSUM->SBUF copy at pair end frees
    the y bank; broadcast matmul + multiply + ytl DMA deferred into the
    next pair's first iterations;
  - x chunk prefetched one chunk ahead (4 split DMAs on the SP ring);
    weights ride the Act HWDGE ring; yts staging on the gpsimd SWDGE
    ring so collective-dependent DMAs never head-block the SP ring.
PSUM banks: pp(2, proj+outproj) + sps(1x2banks, scores) + yps(3, y
accum) + bps(1, 1/l broadcast) = 8.
"""

import sys

sys.path.insert(0, "/opt/trn_rl_repo")

import numpy as np
import ml_dtypes

import concourse.bass as bass
import concourse.mybir as mybir
import concourse.tile as tile

P = 128
B, T, C = 2, 2048, 2048
BT = B * T            # 4096
NH, NKV = 32, 8
HD = C // NH          # 64
G = NH // NKV         # 4 q heads per kv head / per core
CQ = G * HD           # 256 q channels per core
KC = C // P           # 16 contraction chunks
TQ = 512              # t-chunk
NCORES = 8
NTB = BT // TQ        # 8 chunks; chunk tb has (b, qc) = (tb//4, tb%4)
NU = 4                # exchange units: u = 2*b + p, chunk-pair p
UT = P                # t rows per core per unit (64 from each chunk)

f32 = mybir.dt.float32
f32r = mybir.dt.float32r
bf16 = mybir.dt.bfloat16
EXP = mybir.ActivationFunctionType.Exp
SCALE = float(HD) ** -0.5


def split_multi_waits(nc):
    """Walrus codegen allows only one sync-wait per engine instruction; move
    extras onto standalone same-engine EventSemaphore waits placed before."""
    for fn in nc.m.functions:
        for bb in fn.blocks:
            out = []
            for inst in bb.instructions:
                si = inst.sync_info
                if si is not None and si.on_wait and len(si.on_wait) > 1:
                    waits = list(si.on_wait)
                    for j, w in enumerate(waits[:-1]):
                        nop = mybir.InstEventSemaphore(
                            name=f"{inst.name}-ws{j}", ins=[], outs=[],
                            engine=inst.engine)
                        nop.sync_info = mybir.SyncInfo(on_wait=[w], on_update=[])
                        out.append(nop)
                    inst.sync_info = mybir.SyncInfo(
                        on_wait=[waits[-1]], on_update=list(si.on_update))
                out.append(inst)
            try:
                bb.instructions[:] = out
            except TypeError:
                bb.instructions.clear()
                bb.instructions.extend(out)


def build(reps=1, split=True, variant="full"):
    """variant: 'full' | 'nocc' (collectives skipped — timing-only)."""
    nc = bass.Bass(num_devices=NCORES)

    xt_d = nc.dram_tensor("xt", [C, BT], bf16, kind="ExternalInput")
    wq_d = nc.dram_tensor("wq", [C, CQ], bf16, kind="ExternalInput")
    wkv_d = nc.dram_tensor("wkv", [C, P], bf16, kind="ExternalInput")
    wp_d = nc.dram_tensor("wp", [C, C], bf16, kind="ExternalInput")
    bpb_d = nc.dram_tensor("bpb", [P, C], f32, kind="ExternalInput")
    mask_d = nc.dram_tensor("masks", [P, P], bf16, kind="ExternalInput")
    ones_d = nc.dram_tensor("ones", [1, HD], f32r, kind="ExternalInput")
    idn_d = nc.dram_tensor("ident", [P, P], f32, kind="ExternalInput")
    vpad_d = nc.dram_tensor("vpad", [P, (TQ // P) * 2], bf16,
                            kind="ExternalInput")
    out_d = nc.dram_tensor("out", [NU * UT, C], f32, kind="ExternalOutput")

    xt_v = xt_d.rearrange("(o p) n -> p o n", p=P)
    wq_v = wq_d.rearrange("(o p) n -> p o n", p=P)
    wkv_v = wkv_d.rearrange("(o p) n -> p o n", p=P)
    wp_v = wp_d.rearrange("(o p) n -> p o n", p=P)

    with tile.TileContext(nc) as tc:
      for rep in range(reps):
        with tc.tile_pool(name=f"res{rep}", bufs=1) as res, \
             tc.tile_pool(name=f"dram{rep}", bufs=1, space="DRAM") as dp:
            wq_sb = res.tile([P, KC, CQ], bf16, name=f"wq{rep}")
            wkv_sb = res.tile([P, KC, P], bf16, name=f"wkv{rep}")
            wp_sb = res.tile([P, KC, C], bf16, name=f"wp{rep}")
            bpb_sb = res.tile([P, C], f32, name=f"bp{rep}")
            idn_sb = res.tile([P, P], f32, name=f"idn{rep}")
            mask_sb = res.tile([P, P], bf16, name=f"mk{rep}")
            ones_sb = res.tile([1, HD], f32r, name=f"on{rep}")

            # weights & consts on the Act HWDGE ring (SP ring carries x)
            for hh in range(2):
                nc.scalar.dma_start(wq_sb[:, hh * 8:(hh + 1) * 8, :],
                                    wq_v[:, hh * 8:(hh + 1) * 8, :])
            nc.scalar.dma_start(wkv_sb[:], wkv_v[:, :, :])
            nc.scalar.dma_start(idn_sb[:], idn_d[:, :])
            nc.scalar.dma_start(mask_sb[:], mask_d[:, :])
            nc.scalar.dma_start(ones_sb[:], ones_d[:, :])

            qT = [[res.tile([HD, TQ], bf16, name=f"q{rep}_{h}_{tb}")
                   for tb in range(NTB)] for h in range(G)]
            kTt = [res.tile([HD, TQ], bf16, name=f"k{rep}_{tb}")
                   for tb in range(NTB)]
            va_t = [res.tile([P, TQ // P, HD + 2], bf16, name=f"v{rep}_{tb}")
                    for tb in range(NTB)]
            for tb in range(NTB):
                nc.scalar.dma_start(
                    va_t[tb][:, :, HD:HD + 2],
                    vpad_d.rearrange("p (k t) -> p k t", t=2))

            ytl = [dp.tile([NCORES * CQ, UT], bf16, name=f"ytl{rep}_{u}")
                   for u in range(NU)]
            yta = [dp.tile([NCORES * CQ, UT], bf16, name=f"yta{rep}_{u}")
                   for u in range(NU)]

            with tc.tile_pool(name=f"xp{rep}", bufs=2) as xp, \
                 tc.tile_pool(name=f"pp{rep}", bufs=2, space="PSUM") as pp, \
                 tc.tile_pool(name=f"sps{rep}", bufs=1, space="PSUM") as sps, \
                 tc.tile_pool(name=f"yps{rep}", bufs=3, space="PSUM") as yps, \
                 tc.tile_pool(name=f"bps{rep}", bufs=1, space="PSUM") as bps, \
                 tc.tile_pool(name=f"ep{rep}", bufs=3) as ep, \
                 tc.tile_pool(name=f"np{rep}", bufs=3) as npo, \
                 tc.tile_pool(name=f"fp{rep}", bufs=2) as fp, \
                 tc.tile_pool(name=f"vp{rep}", bufs=2) as vp:
                yts_tiles = {}
                xtiles = {}
                pending = []   # deferred norm closures
                fillers = []   # list of (weight, closure): 'light'|'heavy'

                def take_filler(light_only):
                    for i, (w, f) in enumerate(fillers):
                        if not light_only or w == "light":
                            fillers.pop(i)
                            f()
                            return True
                    return False

                def flush_fillers(heavy_only=False):
                    i = 0
                    while i < len(fillers):
                        w, f = fillers[i]
                        if heavy_only and w != "heavy":
                            i += 1
                            continue
                        fillers.pop(i)
                        f()

                def flush_pending():
                    for _, f in pending:
                        f()
                    pending.clear()

                def prefetch_xt(tb):
                    xt_t = xp.tile([P, KC, TQ], bf16, tag="xt",
                                   name=f"xt{rep}_{tb}")
                    for i in range(4):
                        nc.sync.dma_start(
                            xt_t[:, 4 * i:4 * i + 4, :],
                            xt_v[:, 4 * i:4 * i + 4, tb * TQ:(tb + 1) * TQ])
                    xtiles[tb] = xt_t

                def proj_units(tb):
                    """Filler units computing chunk tb's q/k/v projections."""
                    xt_t = xtiles[tb]

                    def qu(half):
                        def f():
                            ps = pp.tile([P, TQ], f32, tag="pp",
                                         name=f"pq{rep}")
                            for c in range(KC):
                                nc.tensor.matmul(
                                    ps[:],
                                    wq_sb[:, c, half * P:(half + 1) * P],
                                    xt_t[:, c, :],
                                    start=(c == 0), stop=(c == KC - 1))
                            nc.vector.tensor_copy(qT[2 * half][tb][:],
                                                  ps[0:HD, :])
                            nc.vector.tensor_copy(qT[2 * half + 1][tb][:],
                                                  ps[HD:P, :])
                        return f

                    vs_box = [None]

                    def kvu():
                        ps = pp.tile([P, TQ], f32, tag="pp", name=f"pk{rep}")
                        for c in range(KC):
                            nc.tensor.matmul(ps[:], wkv_sb[:, c, :],
                                             xt_t[:, c, :],
                                             start=(c == 0),
                                             stop=(c == KC - 1))
                        nc.vector.tensor_copy(kTt[tb][:], ps[0:HD, :])
                        vs = vp.tile([HD, TQ], f32, tag="vs", name=f"vs{rep}")
                        nc.vector.tensor_copy(vs[:], ps[HD:P, :])
                        vs_box[0] = vs

                    def vtu():
                        vs = vs_box[0]
                        for k4 in range(TQ // P):
                            vt_ps = pp.tile([P, HD], f32, tag="pp",
                                            name=f"vt{rep}")
                            nc.tensor.transpose(vt_ps[:],
                                                vs[:, k4 * P:(k4 + 1) * P],
                                                idn_sb[0:HD, 0:HD])
                            nc.vector.tensor_copy(va_t[tb][:, k4, 0:HD],
                                                  vt_ps[:])

                    return [("heavy", qu(0)), ("heavy", qu(1)),
                            ("heavy", kvu), ("heavy", vtu)]

                def emit_norm(b, qc, hp, y0, y1):
                    """recip + PSUM->SBUF copies now (frees the y banks);
                    bc matmul + normalize mul + ytl dma per head deferred."""
                    h0, h1 = 2 * hp, 2 * hp + 1
                    u, c2 = 2 * b + qc // 2, qc % 2
                    rys = []
                    for hi, y_ps in ((0, y0), (1, y1)):
                        rr = npo.tile([1, TQ], f32r, tag=f"rr{hi}",
                                      name=f"rr{rep}")
                        with nc.allow_low_precision(
                                reason="1/l in f32r (22-bit) is plenty"):
                            nc.vector.reciprocal(rr[:], y_ps[HD:HD + 1, :])
                        ys = npo.tile([HD, TQ], bf16, tag=f"ys{hi}",
                                      name=f"ys{rep}")
                        nc.vector.tensor_copy(ys[:], y_ps[0:HD, :])
                        rys.append((rr, ys))

                    def do_head(hi):
                        rr, ys = rys[hi]
                        h = h0 if hi == 0 else h1
                        bc = bps.tile([HD, TQ], f32, tag="bc",
                                      name=f"bc{rep}")
                        nc.tensor.matmul(bc[:], ones_sb[:], rr[:],
                                         start=True, stop=True)
                        yn = npo.tile([HD, TQ], bf16, tag=f"yn{hi}",
                                      name=f"yn{rep}")
                        nc.vector.tensor_mul(yn[:], ys[:], bc[:])
                        dst = ytl[u].rearrange(
                            "(j ch) (c2 t) -> ch j c2 t", j=NCORES, c2=2)
                        nc.sync.dma_start(
                            dst[h * HD:(h + 1) * HD, :, c2, :],
                            yn[:].rearrange("d (j t) -> d j t", j=NCORES))

                    pending.append(("h0", lambda: do_head(0)))
                    pending.append(("h1", lambda: do_head(1)))

                def emit_exchange(u):
                    if variant != "nocc":
                        nc.gpsimd.collective_compute(
                            "AllToAll", mybir.AluOpType.bypass,
                            replica_groups=[list(range(NCORES))],
                            ins=[ytl[u][:].opt()], outs=[yta[u][:].opt()])
                    yts = fp.tile([P, KC, UT], bf16, tag="yt",
                                  name=f"yt{rep}")
                    nc.gpsimd.dma_start(
                        yts[:], yta[u].rearrange("(c p) t -> p c t", p=P))
                    yts_tiles[u] = yts

                def outproj_units(u):
                    yts = yts_tiles.pop(u)

                    def ou(occ):
                        def f():
                            o_ps = pp.tile([P, TQ], f32, tag="pp",
                                           name=f"o{rep}_{u}_{occ}")
                            for c in range(KC):
                                nc.tensor.matmul(
                                    o_ps[:], yts[:, c, :],
                                    wp_sb[:, c, occ * TQ:(occ + 1) * TQ],
                                    start=(c == 0), stop=(c == KC - 1))
                            o_sb = fp.tile([P, TQ], f32, tag="ob",
                                           name=f"ob{rep}")
                            nc.vector.tensor_add(
                                o_sb[:], o_ps[:],
                                bpb_sb[:, occ * TQ:(occ + 1) * TQ])
                            nc.sync.dma_start(
                                out_d[u * P:(u + 1) * P,
                                      occ * TQ:(occ + 1) * TQ],
                                o_sb[:])
                        return f

                    return [("light", ou(occ)) for occ in range(4)]

                # ---------------- chunk loop ----------------
                prefetch_xt(0)
                for tb in range(NTB):
                    b, qc = tb // 4, tb % 4
                    if tb + 1 < NTB:
                        prefetch_xt(tb + 1)
                    if tb == 0:
                        # chunk 0's projections run inline (nothing to
                        # overlap them with yet)
                        for _, f in proj_units(0):
                            f()
                    if tb + 1 < NTB:
                        fillers.extend(proj_units(tb + 1))
                    if tb == 3:
                        fillers.extend(outproj_units(0))
                    elif tb == 5:
                        fillers.extend(outproj_units(1))
                    elif tb == 7:
                        fillers.extend(outproj_units(2))

                    # ---- attention: head pairs, merged-exp pipeline ----
                    nkb = 4 * qc + 4
                    for hp in range(2):
                        h0, h1 = 2 * hp, 2 * hp + 1
                        y0 = yps.tile([HD + 2, TQ], f32, tag="y",
                                      name=f"y{rep}_{tb}_{h0}")
                        y1 = yps.tile([HD + 2, TQ], f32, tag="y",
                                      name=f"y{rep}_{tb}_{h1}")
                        prev = [None]   # (ex2, kb, qoff)

                        def emit_avs(y0=y0, y1=y1, nkb=nkb, b=b, prev=prev):
                            ex2, kb, qoff = prev[0]
                            tb_k = b * 4 + kb // 4
                            for hi, y_ps in ((0, y0), (1, y1)):
                                nc.tensor.matmul(
                                    y_ps[:, qoff:TQ],
                                    va_t[tb_k][:, kb % 4, :],
                                    ex2[:, hi, qoff:TQ],
                                    start=(kb == 0), stop=(kb == nkb - 1))
                            prev[0] = None

                        for kb in range(nkb):
                            j = kb - 4 * qc
                            qoff = max(0, j * P)
                            fr = TQ - qoff
                            tb_k = b * 4 + kb // 4
                            s2 = sps.tile([P, 2, TQ], f32, tag="s2",
                                          name=f"s{rep}")
                            for hi, h in ((0, h0), (1, h1)):
                                nc.tensor.matmul(
                                    s2[:, hi, 0:fr],
                                    kTt[tb_k][:, (kb % 4) * P:
                                              (kb % 4 + 1) * P],
                                    qT[h][tb][:, qoff:TQ],
                                    start=True, stop=True)
                            ex2 = ep.tile([P, 2, TQ], bf16, tag="ex",
                                          name=f"ex{rep}")
                            nc.scalar.activation(ex2[:, :, qoff:TQ],
                                                 s2[:, :, 0:fr], EXP,
                                                 scale=SCALE)
                            if j >= 0:
                                for hi in range(2):
                                    nc.vector.tensor_mul(
                                        ex2[:, hi, qoff:qoff + P],
                                        ex2[:, hi, qoff:qoff + P],
                                        mask_sb[:])
                            # norm-of-prev-pair injections (y-bank order)
                            if kb == 0 and pending:
                                pending[0][1]()
                                del pending[0]
                            if kb == 1 and pending:
                                flush_pending()
                            if prev[0] is not None:
                                emit_avs()
                            # filler slot
                            it_global = hp * nkb + kb
                            can_heavy = (hp == 1) or \
                                (it_global >= max(4, nkb // 2))
                            if j >= 1:
                                take_filler(light_only=not (qc == 0 and
                                                            can_heavy))
                            else:
                                take_filler(light_only=not can_heavy)
                            prev[0] = (ex2, kb, qoff)
                        emit_avs()
                        emit_norm(b, qc, hp, y0, y1)

                    flush_fillers(heavy_only=True)
                    # wp spread over chunks 0-1 on the Act ring (out-proj
                    # of unit 0 consumes it from chunk 3)
                    if tb < 2:
                        for ww in range(2):
                            wc = 8 * tb + 4 * ww
                            nc.scalar.dma_start(
                                wp_sb[:, wc:wc + 4, :],
                                wp_v[:, wc:wc + 4, :])
                    if tb == 1:
                        nc.scalar.dma_start(bpb_sb[:], bpb_d[:, :])

                    # ---- unit boundaries ----
                    if tb % 2 == 1:
                        flush_pending()
                        emit_exchange(2 * b + qc // 2)
                    if tb == 7:
                        flush_fillers()
                        for _, f in outproj_units(3):
                            f()

    if split:
        split_multi_waits(nc)
    return nc


_NC_CACHE = None


def _get_nc():
    global _NC_CACHE
    if _NC_CACHE is None:
        _NC_CACHE = build()
    return _NC_CACHE


def make_in_maps(x, wq, wk, wv, wp, bp):
    x = np.asarray(x, dtype=np.float32)
    xt = np.ascontiguousarray(x.reshape(BT, C).T).astype(ml_dtypes.bfloat16)
    wp_b = np.ascontiguousarray(np.asarray(wp, np.float32)).astype(
        ml_dtypes.bfloat16)
    bpb = np.tile(np.asarray(bp, np.float32)[None, :], (P, 1))
    mask = np.triu(np.ones((P, P), np.float32)).astype(ml_dtypes.bfloat16)
    ident = np.eye(P, dtype=np.float32)
    vpad = np.zeros((P, TQ // P, 2), np.float32)
    vpad[:, :, 0] = 1.0
    vpad = vpad.reshape(P, -1).astype(ml_dtypes.bfloat16)
    in_maps = []
    for i in range(NCORES):
        cs = slice(i * CQ, (i + 1) * CQ)
        ks = slice(i * HD, (i + 1) * HD)
        wkv = np.concatenate(
            [np.asarray(wk, np.float32)[:, ks],
             np.asarray(wv, np.float32)[:, ks]], axis=1)
        in_maps.append({
            "xt": xt,
            "wq": np.ascontiguousarray(
                np.asarray(wq, np.float32)[:, cs]).astype(ml_dtypes.bfloat16),
            "wkv": np.ascontiguousarray(wkv).astype(ml_dtypes.bfloat16),
            "wp": wp_b,
            "bpb": bpb,
            "masks": mask,
            "ones": np.ones((1, HD), np.float32),
            "ident": ident,
            "vpad": vpad,
        })
    return in_maps


def kernel(x, wq, wk, wv, wp, bp, _trace=False):
    from concourse.bass_utils import run_bass_kernel_spmd
    nc = _get_nc()
    in_maps = make_in_maps(x, wq, wk, wv, wp, bp)
    res = run_bass_kernel_spmd(nc, in_maps, list(range(NCORES)), trace=_trace)
    out = np.empty((B, T, C), np.float32)
    for i in range(NCORES):
        o = res.results[i]["out"]       # [NU*UT, C]
        for u in range(NU):
            bb, p = u // 2, u % 2
            for c2 in range(2):
                t0 = (2 * p + c2) * TQ + i * HD
                out[bb, t0:t0 + HD, :] = \
                    o[u * P + c2 * HD:u * P + (c2 + 1) * HD, :]
    if _trace:
        return out, res
    return out


# revision 13
# speedup vs baseline: 3.1206x; 2.1656x over previous
"""GQA causal attention (B=2, T=2048, C=2048, 32 Q heads, 8 KV heads) on 8
Trainium2 NeuronCores — v6.

Sharding: tensor-parallel over KV-head groups for projections+attention
(core i owns KV head i and its 4 query heads). Output projection is
row-sharded with INTERLEAVED ownership: for each exchange unit
u = (batch b, chunk-pair p) covering t-chunks qc=2p,2p+1, core i owns the
i-th 64-t slice of each chunk. One AllToAll per unit (4 total, 512 KiB
per core each) so only the last unit's exchange + one 128-row out-proj
sit after the final attention chunk. Collectives measured free on HW
(fully overlapped).

v6 structure (attention on HW is Act-engine bound: exp costs
(N+352)/1.2 ns, i.e. 720 ns per [128,512] block vs 852 ns of PE work
per 2-head iteration):
  - both heads of a pair score into ONE 2-bank PSUM tile [128,2,512];
    a single activation with a 3D AP exps both (N=2*fr amortizes the
    352-cycle fixed cost): Act total 210 -> ~163 us, under PE's ~270;
  - pipelined inner loop: iteration kb emits scores(kb) (gated on
    exp(kb-1) freeing the score tile) then AVs(kb-1) from the previous
    ex tile, so the PE never waits exp latency in-line;
  - FILLER injection: next chunk's projection groups and ready
    out-projection occ-groups are injected into attention iterations,
    giving the PE work while Act catches up and covering the thin
    diagonal blocks; light (out-proj) fillers carry across chunk
    boundaries, projection fillers flush before their chunk's attention;
  - AV matmuls trimmed to the causal range [qoff:] (no memsets);
  - per-head normalization: recip + PSUM->SBUF copy at pair end frees
    the y bank; broadcast matmul + multiply + ytl DMA deferred into the
    next pair's first iterations;
  - x chunk prefetched one chunk ahead (4 split DMAs on the SP ring);
    weights ride the Act HWDGE ring; yts staging on the gpsimd SWDGE
    ring so collective-dependent DMAs never head-block the SP ring.
PSUM banks: pp(2, proj+outproj) + sps(1x2banks, scores) + yps(3, y
accum) + bps(1, 1/l broadcast) = 8.
"""

import sys

sys.path.insert(0, "/opt/trn_rl_repo")

import numpy as np
import ml_dtypes

import concourse.bass as bass
import concourse.mybir as mybir
import concourse.tile as tile

P = 128
B, T, C = 2, 2048, 2048
BT = B * T            # 4096
NH, NKV = 32, 8
HD = C // NH          # 64
G = NH // NKV         # 4 q heads per kv head / per core
CQ = G * HD           # 256 q channels per core
KC = C // P           # 16 contraction chunks
TQ = 512              # t-chunk
NCORES = 8
NTB = BT // TQ        # 8 chunks; chunk tb has (b, qc) = (tb//4, tb%4)
NU = 4                # exchange units: u = 2*b + p, chunk-pair p
UT = P                # t rows per core per unit (64 from each chunk)

f32 = mybir.dt.float32
f32r = mybir.dt.float32r
bf16 = mybir.dt.bfloat16
EXP = mybir.ActivationFunctionType.Exp
SCALE = float(HD) ** -0.5


def split_multi_waits(nc):
    """Walrus codegen allows only one sync-wait per engine instruction; move
    extras onto standalone same-engine EventSemaphore waits placed before."""
    for fn in nc.m.functions:
        for bb in fn.blocks:
            out = []
            for inst in bb.instructions:
                si = inst.sync_info
                if si is not None and si.on_wait and len(si.on_wait) > 1:
                    waits = list(si.on_wait)
                    for j, w in enumerate(waits[:-1]):
                        nop = mybir.InstEventSemaphore(
                            name=f"{inst.name}-ws{j}", ins=[], outs=[],
                            engine=inst.engine)
                        nop.sync_info = mybir.SyncInfo(on_wait=[w], on_update=[])
                        out.append(nop)
                    inst.sync_info = mybir.SyncInfo(
                        on_wait=[waits[-1]], on_update=list(si.on_update))
                out.append(inst)
            try:
                bb.instructions[:] = out
            except TypeError:
                bb.instructions.clear()
                bb.instructions.extend(out)


def build(reps=1, split=True, variant="full"):
    """variant: 'full' | 'nocc' (collectives skipped — timing-only)."""
    nc = bass.Bass(num_devices=NCORES)

    xt_d = nc.dram_tensor("xt", [C, BT], bf16, kind="ExternalInput")
    wq_d = nc.dram_tensor("wq", [C, CQ], bf16, kind="ExternalInput")
    wkv_d = nc.dram_tensor("wkv", [C, P], bf16, kind="ExternalInput")
    wp_d = nc.dram_tensor("wp", [C, C], bf16, kind="ExternalInput")
    bpb_d = nc.dram_tensor("bpb", [P, C], f32, kind="ExternalInput")
    mask_d = nc.dram_tensor("masks", [P, P], bf16, kind="ExternalInput")
    ones_d = nc.dram_tensor("ones", [1, HD], f32r, kind="ExternalInput")
    idn_d = nc.dram_tensor("ident", [P, P], f32, kind="ExternalInput")
    vpad_d = nc.dram_tensor("vpad", [P, (TQ // P) * 2], bf16,
                            kind="ExternalInput")
    out_d = nc.dram_tensor("out", [NU * UT, C], f32, kind="ExternalOutput")

    xt_v = xt_d.rearrange("(o p) n -> p o n", p=P)
    wq_v = wq_d.rearrange("(o p) n -> p o n", p=P)
    wkv_v = wkv_d.rearrange("(o p) n -> p o n", p=P)
    wp_v = wp_d.rearrange("(o p) n -> p o n", p=P)

    with tile.TileContext(nc) as tc:
      for rep in range(reps):
        with tc.tile_pool(name=f"res{rep}", bufs=1) as res, \
             tc.tile_pool(name=f"dram{rep}", bufs=1, space="DRAM") as dp:
            wq_sb = res.tile([P, KC, CQ], bf16, name=f"wq{rep}")
            wkv_sb = res.tile([P, KC, P], bf16, name=f"wkv{rep}")
            wp_sb = res.tile([P, KC, C], bf16, name=f"wp{rep}")
            bpb_sb = res.tile([P, C], f32, name=f"bp{rep}")
            idn_sb = res.tile([P, P], f32, name=f"idn{rep}")
            mask_sb = res.tile([P, P], bf16, name=f"mk{rep}")
            ones_sb = res.tile([1, HD], f32r, name=f"on{rep}")

            # weights & consts on the Act HWDGE ring (SP ring carries x)
            for hh in range(2):
                nc.scalar.dma_start(wq_sb[:, hh * 8:(hh + 1) * 8, :],
                                    wq_v[:, hh * 8:(hh + 1) * 8, :])
            nc.scalar.dma_start(wkv_sb[:], wkv_v[:, :, :])
            nc.scalar.dma_start(idn_sb[:], idn_d[:, :])
            nc.scalar.dma_start(mask_sb[:], mask_d[:, :])
            nc.scalar.dma_start(ones_sb[:], ones_d[:, :])

            qT = [[res.tile([HD, TQ], bf16, name=f"q{rep}_{h}_{tb}")
                   for tb in range(NTB)] for h in range(G)]
            kTt = [res.tile([HD, TQ], bf16, name=f"k{rep}_{tb}")
                   for tb in range(NTB)]
            va_t = [res.tile([P, TQ // P, HD + 2], bf16, name=f"v{rep}_{tb}")
                    for tb in range(NTB)]
            for tb in range(NTB):
                nc.scalar.dma_start(
                    va_t[tb][:, :, HD:HD + 2],
                    vpad_d.rearrange("p (k t) -> p k t", t=2))

            ytl = [dp.tile([NCORES * CQ, UT], bf16, name=f"ytl{rep}_{u}")
                   for u in range(NU)]
            yta = [dp.tile([NCORES * CQ, UT], bf16, name=f"yta{rep}_{u}")
                   for u in range(NU)]

            with tc.tile_pool(name=f"xp{rep}", bufs=2) as xp, \
                 tc.tile_pool(name=f"pp{rep}", bufs=2, space="PSUM") as pp, \
                 tc.tile_pool(name=f"sps{rep}", bufs=1, space="PSUM") as sps, \
                 tc.tile_pool(name=f"yps{rep}", bufs=3, space="PSUM") as yps, \
                 tc.tile_pool(name=f"bps{rep}", bufs=1, space="PSUM") as bps, \
                 tc.tile_pool(name=f"ep{rep}", bufs=3) as ep, \
                 tc.tile_pool(name=f"np{rep}", bufs=3) as npo, \
                 tc.tile_pool(name=f"fp{rep}", bufs=2) as fp, \
                 tc.tile_pool(name=f"vp{rep}", bufs=2) as vp:
                yts_tiles = {}
                xtiles = {}
                pending = []   # deferred norm closures
                fillers = []   # list of (weight, closure): 'light'|'heavy'

                def take_filler(light_only):
                    for i, (w, f) in enumerate(fillers):
                        if not light_only or w == "light":
                            fillers.pop(i)
                            f()
                            return True
                    return False

                def flush_fillers(heavy_only=False):
                    i = 0
                    while i < len(fillers):
                        w, f = fillers[i]
                        if heavy_only and w != "heavy":
                            i += 1
                            continue
                        fillers.pop(i)
                        f()

                def flush_pending():
                    for _, f in pending:
                        f()
                    pending.clear()

                def prefetch_xt(tb):
                    xt_t = xp.tile([P, KC, TQ], bf16, tag="xt",
                                   name=f"xt{rep}_{tb}")
                    for i in range(4):
                        nc.sync.dma_start(
                            xt_t[:, 4 * i:4 * i + 4, :],
                            xt_v[:, 4 * i:4 * i + 4, tb * TQ:(tb + 1) * TQ])
                    xtiles[tb] = xt_t

                def proj_units(tb):
                    """Filler units computing chunk tb's q/k/v projections."""
                    xt_t = xtiles[tb]

                    def qu(half):
                        def f():
                            ps = pp.tile([P, TQ], f32, tag="pp",
                                         name=f"pq{rep}")
                            for c in range(KC):
                                nc.tensor.matmul(
                                    ps[:],
                                    wq_sb[:, c, half * P:(half + 1) * P],
                                    xt_t[:, c, :],
                                    start=(c == 0), stop=(c == KC - 1))
                            nc.vector.tensor_copy(qT[2 * half][tb][:],
                                                  ps[0:HD, :])
                            nc.vector.tensor_copy(qT[2 * half + 1][tb][:],
                                                  ps[HD:P, :])
                        return f

                    vs_box = [None]

                    def kvu():
                        ps = pp.tile([P, TQ], f32, tag="pp", name=f"pk{rep}")
                        for c in range(KC):
                            nc.tensor.matmul(ps[:], wkv_sb[:, c, :],
                                             xt_t[:, c, :],
                                             start=(c == 0),
                                             stop=(c == KC - 1))
                        nc.vector.tensor_copy(kTt[tb][:], ps[0:HD, :])
                        vs = vp.tile([HD, TQ], f32, tag="vs", name=f"vs{rep}")
                        nc.vector.tensor_copy(vs[:], ps[HD:P, :])
                        vs_box[0] = vs

                    def vtu():
                        vs = vs_box[0]
                        for k4 in range(TQ // P):
                            vt_ps = pp.tile([P, HD], f32, tag="pp",
                                            name=f"vt{rep}")
                            nc.tensor.transpose(vt_ps[:],
                                                vs[:, k4 * P:(k4 + 1) * P],
                                                idn_sb[0:HD, 0:HD])
                            nc.vector.tensor_copy(va_t[tb][:, k4, 0:HD],
                                                  vt_ps[:])

                    return [("heavy", qu(0)), ("heavy", qu(1)),
                            ("heavy", kvu), ("heavy", vtu)]

                def emit_norm(b, qc, hp, y0, y1):
                    """recip + PSUM->SBUF copies now (frees the y banks);
                    bc matmul + normalize mul + ytl dma per head deferred."""
                    h0, h1 = 2 * hp, 2 * hp + 1
                    u, c2 = 2 * b + qc // 2, qc % 2
                    rys = []
                    for hi, y_ps in ((0, y0), (1, y1)):
                        rr = npo.tile([1, TQ], f32r, tag=f"rr{hi}",
                                      name=f"rr{rep}")
                        with nc.allow_low_precision(
                                reason="1/l in f32r (22-bit) is plenty"):
                            nc.vector.reciprocal(rr[:], y_ps[HD:HD + 1, :])
                        ys = npo.tile([HD, TQ], bf16, tag=f"ys{hi}",
                                      name=f"ys{rep}")
                        nc.vector.tensor_copy(ys[:], y_ps[0:HD, :])
                        rys.append((rr, ys))

                    def do_head(hi):
                        rr, ys = rys[hi]
                        h = h0 if hi == 0 else h1
                        bc = bps.tile([HD, TQ], f32, tag="bc",
                                      name=f"bc{rep}")
                        nc.tensor.matmul(bc[:], ones_sb[:], rr[:],
                                         start=True, stop=True)
                        yn = npo.tile([HD, TQ], bf16, tag=f"yn{hi}",
                                      name=f"yn{rep}")
                        nc.vector.tensor_mul(yn[:], ys[:], bc[:])
                        dst = ytl[u].rearrange(
                            "(j ch) (c2 t) -> ch j c2 t", j=NCORES, c2=2)
                        nc.sync.dma_start(
                            dst[h * HD:(h + 1) * HD, :, c2, :],
                            yn[:].rearrange("d (j t) -> d j t", j=NCORES))

                    pending.append(("h0", lambda: do_head(0)))
                    pending.append(("h1", lambda: do_head(1)))

                def emit_exchange(u):
                    if variant != "nocc":
                        nc.gpsimd.collective_compute(
                            "AllToAll", mybir.AluOpType.bypass,
                            replica_groups=[list(range(NCORES))],
                            ins=[ytl[u][:].opt()], outs=[yta[u][:].opt()])
                    yts = fp.tile([P, KC, UT], bf16, tag="yt",
                                  name=f"yt{rep}")
                    nc.gpsimd.dma_start(
                        yts[:], yta[u].rearrange("(c p) t -> p c t", p=P))
                    yts_tiles[u] = yts

                def outproj_units(u):
                    yts = yts_tiles.pop(u)

                    def ou(occ):
                        def f():
                            o_ps = pp.tile([P, TQ], f32, tag="pp",
                                           name=f"o{rep}_{u}_{occ}")
                            for c in range(KC):
                                nc.tensor.matmul(
                                    o_ps[:], yts[:, c, :],
                                    wp_sb[:, c, occ * TQ:(occ + 1) * TQ],
                                    start=(c == 0), stop=(c == KC - 1))
                            o_sb = fp.tile([P, TQ], f32, tag="ob",
                                           name=f"ob{rep}")
                            nc.vector.tensor_add(
                                o_sb[:], o_ps[:],
                                bpb_sb[:, occ * TQ:(occ + 1) * TQ])
                            nc.sync.dma_start(
                                out_d[u * P:(u + 1) * P,
                                      occ * TQ:(occ + 1) * TQ],
                                o_sb[:])
                        return f

                    return [("light", ou(occ)) for occ in range(4)]

                # ---------------- chunk loop ----------------
                prefetch_xt(0)
                for tb in range(NTB):
                    b, qc = tb // 4, tb % 4
                    if tb + 1 < NTB:
                        prefetch_xt(tb + 1)
                    if tb == 0:
                        # chunk 0's projections run inline (nothing to
                        # overlap them with yet)
                        for _, f in proj_units(0):
                            f()
                    if tb + 1 < NTB:
                        fillers.extend(proj_units(tb + 1))
                    if tb == 3:
                        fillers.extend(outproj_units(0))
                    elif tb == 5:
                        fillers.extend(outproj_units(1))
                    elif tb == 7:
                        fillers.extend(outproj_units(2))

                    # ---- attention: head pairs, merged-exp pipeline ----
                    nkb = 4 * qc + 4
                    for hp in range(2):
                        h0, h1 = 2 * hp, 2 * hp + 1
                        y0 = yps.tile([HD + 2, TQ], f32, tag="y",
                                      name=f"y{rep}_{tb}_{h0}")
                        y1 = yps.tile([HD + 2, TQ], f32, tag="y",
                                      name=f"y{rep}_{tb}_{h1}")
                        prev = [None]   # (ex2, kb, qoff)

                        def emit_avs(y0=y0, y1=y1, nkb=nkb, b=b, prev=prev):
                            ex2, kb, qoff = prev[0]
                            tb_k = b * 4 + kb // 4
                            for hi, y_ps in ((0, y0), (1, y1)):
                                nc.tensor.matmul(
                                    y_ps[:, qoff:TQ],
                                    va_t[tb_k][:, kb % 4, :],
                                    ex2[:, hi, qoff:TQ],
                                    start=(kb == 0), stop=(kb == nkb - 1))
                            prev[0] = None

                        for kb in range(nkb):
                            j = kb - 4 * qc
                            qoff = max(0, j * P)
                            fr = TQ - qoff
                            tb_k = b * 4 + kb // 4
                            s2 = sps.tile([P, 2, TQ], f32, tag="s2",
                                          name=f"s{rep}")
                            for hi, h in ((0, h0), (1, h1)):
                                nc.tensor.matmul(
                                    s2[:, hi, 0:fr],
                                    kTt[tb_k][:, (kb % 4) * P:
                                              (kb % 4 + 1) * P],
                                    qT[h][tb][:, qoff:TQ],
                                    start=True, stop=True)
                            ex2 = ep.tile([P, 2, TQ], bf16, tag="ex",
                                          name=f"ex{rep}")
                            nc.scalar.activation(ex2[:, :, qoff:TQ],
                                                 s2[:, :, 0:fr], EXP,
                                                 scale=SCALE)
                            if j >= 0:
                                for hi in range(2):
                                    nc.vector.tensor_mul(
                                        ex2[:, hi, qoff:qoff + P],
                                        ex2[:, hi, qoff:qoff + P],
                                        mask_sb[:])
                            # norm-of-prev-pair injections (y-bank order)
                            if kb == 0 and pending:
                                pending[0][1]()
                                del pending[0]
                            if kb == 1 and pending:
                                flush_pending()
                            if prev[0] is not None:
                                emit_avs()
                            # filler slot
                            it_global = hp * nkb + kb
                            can_heavy = (hp == 1) or \
                                (it_global >= max(4, nkb // 2))
                            if j >= 1:
                                take_filler(light_only=not (qc == 0 and
                                                            can_heavy))
                            else:
                                take_filler(light_only=not can_heavy)
                            prev[0] = (ex2, kb, qoff)
                        emit_avs()
                        emit_norm(b, qc, hp, y0, y1)

                    flush_fillers(heavy_only=True)
                    # wp spread over chunks 0-1 on the Act ring (out-proj
                    # of unit 0 consumes it from chunk 3)
                    if tb < 2:
                        for ww in range(2):
                            wc = 8 * tb + 4 * ww
                            nc.scalar.dma_start(
                                wp_sb[:, wc:wc + 4, :],
                                wp_v[:, wc:wc + 4, :])
                    if tb == 1:
                        nc.scalar.dma_start(bpb_sb[:], bpb_d[:, :])

                    # ---- unit boundaries ----
                    if tb % 2 == 1:
                        flush_pending()
                        emit_exchange(2 * b + qc // 2)
                    if tb == 7:
                        flush_fillers()
                        for _, f in outproj_units(3):
                            f()

    if split:
        split_multi_waits(nc)
    return nc


_NC_CACHE = None


def _get_nc():
    global _NC_CACHE
    if _NC_CACHE is None:
        _NC_CACHE = build()
    return _NC_CACHE


def make_in_maps(x, wq, wk, wv, wp, bp):
    x = np.asarray(x, dtype=np.float32)
    xt = np.ascontiguousarray(x.reshape(BT, C).T).astype(ml_dtypes.bfloat16)
    wp_b = np.ascontiguousarray(np.asarray(wp, np.float32)).astype(
        ml_dtypes.bfloat16)
    bpb = np.tile(np.asarray(bp, np.float32)[None, :], (P, 1))
    mask = np.triu(np.ones((P, P), np.float32)).astype(ml_dtypes.bfloat16)
    ident = np.eye(P, dtype=np.float32)
    vpad = np.zeros((P, TQ // P, 2), np.float32)
    vpad[:, :, 0] = 1.0
    vpad = vpad.reshape(P, -1).astype(ml_dtypes.bfloat16)
    in_maps = []
    for i in range(NCORES):
        cs = slice(i * CQ, (i + 1) * CQ)
        ks = slice(i * HD, (i + 1) * HD)
        wkv = np.concatenate(
            [np.asarray(wk, np.float32)[:, ks],
             np.asarray(wv, np.float32)[:, ks]], axis=1)
        in_maps.append({
            "xt": xt,
            "wq": np.ascontiguousarray(
                np.asarray(wq, np.float32)[:, cs]).astype(ml_dtypes.bfloat16),
            "wkv": np.ascontiguousarray(wkv).astype(ml_dtypes.bfloat16),
            "wp": wp_b,
            "bpb": bpb,
            "masks": mask,
            "ones": np.ones((1, HD), np.float32),
            "ident": ident,
            "vpad": vpad,
        })
    return in_maps


def kernel(x, wq, wk, wv, wp, bp, _trace=False):
    from concourse.bass_utils import run_bass_kernel_spmd
    nc = _get_nc()
    in_maps = make_in_maps(x, wq, wk, wv, wp, bp)
    res = run_bass_kernel_spmd(nc, in_maps, list(range(NCORES)), trace=_trace)
    out = np.empty((B, T, C), np.float32)
    for i in range(NCORES):
        o = res.results[i]["out"]       # [NU*UT, C]
        for u in range(NU):
            bb, p = u // 2, u % 2
            for c2 in range(2):
                t0 = (2 * p + c2) * TQ + i * HD
                out[bb, t0:t0 + HD, :] = \
                    o[u * P + c2 * HD:u * P + (c2 + 1) * HD, :]
    if _trace:
        return out, res
    return out
